# revision 13
# baseline (speedup 1.0000x reference)
"""Trainium2 Bass kernel for nn_PlanLayer: batched factor-graph GN step.

Per trajectory: bilinear SDF obstacle factors + GP chain factors ->
block-tridiagonal (4x4 blocks, 128 states) normal equations -> cyclic
reduction solve. Data parallel: 16 trajectories/core x 8 cores.
On-chip layout: 128 partitions = 16 batches x 8 lanes; a lane owns 16
consecutive states of its batch's chain.
"""
import numpy as np
import concourse.bass as bass
import concourse.bacc as bacc
import concourse.mybir as mybir
from concourse.tile import TileContext
import itertools
_TAGC = itertools.count()


def _mt(pool, shape, dtype=None, tag=None):
    import concourse.mybir as _mb
    n = f"t{next(_TAGC)}"
    return pool.tile(shape, dtype or _mb.dt.float32, name=n, tag=tag or n)

f32 = mybir.dt.float32
AL = mybir.AluOpType
AF = mybir.ActivationFunctionType

P = 128
BL = 16          # batches per core
T1 = 128
SD = 4
NGP = T1 - 1
DT = 5.0 / NGP
WS = HS = 128
KSI = 1e4        # 1/sigma^2 for start/goal priors
REG = 1e-3

SM = np.array([[12.0 / DT**3, -6.0 / DT**2], [-6.0 / DT**2, 4.0 / DT]], np.float64)
PM = np.array([[1.0, DT], [0.0, 1.0]], np.float64)
PSP = PM.T @ SM @ PM
PTS = PM.T @ SM


def _ap(tile, pc, off, dims):
    b = tile[:]
    return bass.AP(b.tensor, b.offset + off,
                   [[b.ap[0][0], pc]] + [list(d) for d in dims])


def _dap(dram, off, dims):
    b = dram[:]
    return bass.AP(b.tensor, b.offset + off, [list(d) for d in dims])


class Mat:
    def __init__(self, tile, off, dm, dr, dc):
        self.tile, self.off, self.dm, self.dr, self.dc = tile, off, dm, dr, dc

    def T(self):
        return Mat(self.tile, self.off, self.dm, self.dc, self.dr)


class Vec:
    def __init__(self, tile, off, dm, de):
        self.tile, self.off, self.dm, self.de = tile, off, dm, de


class Emit:
    """Batched small-matrix op emitter. n = sets per partition."""

    def __init__(self, nc, pool, pc):
        self.nc, self.pool, self.pc = nc, pool, pc
        self.n = 1
        self._ms = itertools.cycle([f"ms{i}" for i in range(4)])
        self._mp = itertools.cycle([f"mp{i}" for i in range(8)])
        self._mv = itertools.cycle([f"mv{i}" for i in range(6)])

    def mm44(self, A, B, negate=False, tag=None):
        nc, pool, pc, n = self.nc, self.pool, self.pc, self.n
        scr = _mt(pool, [P, 4 * n * 16], tag=next(self._ms))
        prod = _mt(pool, [P, n * 16], tag=tag or next(self._mp))
        for i in range(4):
            nc.vector.tensor_tensor(
                out=_ap(scr, pc, i * n * 16, [(16, n), (4, 4), (1, 4)]),
                in0=_ap(A.tile, pc, A.off + i * A.dr, [(A.dm, n), (0, 4), (A.dc, 4)]),
                in1=_ap(B.tile, pc, B.off, [(B.dm, n), (B.dc, 4), (B.dr, 4)]),
                op=AL.mult)
        nc.vector.tensor_reduce(
            _ap(prod, pc, 0, [(1, 4 * n * 4)]),
            _ap(scr, pc, 0, [(4, 4 * n * 4), (1, 4)]),
            mybir.AxisListType.X, AL.add, negate=negate)
        return Mat(prod, 0, 4, n * 4, 1)

    def mv4(self, A, v, out=None):
        nc, pool, pc, n = self.nc, self.pool, self.pc, self.n
        scr = _mt(pool, [P, n * 16], tag=next(self._ms))
        nc.vector.tensor_tensor(
            out=_ap(scr, pc, 0, [(16, n), (4, 4), (1, 4)]),
            in0=_ap(A.tile, pc, A.off, [(A.dm, n), (A.dr, 4), (A.dc, 4)]),
            in1=_ap(v.tile, pc, v.off, [(v.dm, n), (0, 4), (v.de, 4)]),
            op=AL.mult)
        if out is None:
            res = _mt(pool, [P, n * 4], tag=next(self._mv))
            nc.vector.tensor_reduce(
                _ap(res, pc, 0, [(1, n * 4)]),
                _ap(scr, pc, 0, [(4, n * 4), (1, 4)]),
                mybir.AxisListType.X, AL.add)
            return Vec(res, 0, 4, 1)
        nc.vector.tensor_reduce(
            out, _ap(scr, pc, 0, [(4, n * 4), (1, 4)]),
            mybir.AxisListType.X, AL.add)
        return None

    def inv44(self, C, Mout, sgn4):
        nc, pool, pc, n = self.nc, self.pool, self.pc, self.n
        _ivc = itertools.count()

        def t(sz):
            return _mt(pool, [P, sz], tag=f"iv{next(_ivc)}")

        def inv22(Ps, store22):
            d2 = t(n * 2)
            nc.vector.tensor_tensor(
                out=_ap(d2, pc, 0, [(2, n), (1, 2)]),
                in0=_ap(Ps.tile, pc, Ps.off, [(Ps.dm, n), (Ps.dc, 2)]),
                in1=_ap(Ps.tile, pc, Ps.off + Ps.dr + Ps.dc,
                        [(Ps.dm, n), (-Ps.dc, 2)]),
                op=AL.mult)
            dd = t(n)
            nc.vector.tensor_tensor(
                out=_ap(dd, pc, 0, [(1, n)]),
                in0=_ap(d2, pc, 0, [(2, n)]),
                in1=_ap(d2, pc, 1, [(2, n)]),
                op=AL.subtract)
            rd = t(n)
            nc.vector.reciprocal(_ap(rd, pc, 0, [(1, n)]), _ap(dd, pc, 0, [(1, n)]))
            pv = t(n * 4)
            nc.vector.tensor_tensor(
                out=_ap(pv, pc, 0, [(4, n), (2, 2), (1, 2)]),
                in0=_ap(Ps.tile, pc, Ps.off + Ps.dr + Ps.dc,
                        [(Ps.dm, n), (-Ps.dr, 2), (-Ps.dc, 2)]),
                in1=_ap(sgn4, pc, 0, [(0, n), (2, 2), (1, 2)]),
                op=AL.mult)
            nc.vector.tensor_tensor(
                out=_ap(store22, pc, 0, [(4, n), (1, 4)]),
                in0=_ap(pv, pc, 0, [(4, n), (1, 4)]),
                in1=_ap(rd, pc, 0, [(1, n), (0, 4)]),
                op=AL.mult)
            return Mat(store22, 0, 4, 2, 1)

        def mm22(A2, B2, store):
            s0 = t(n * 4)
            s1 = t(n * 4)
            for k, s in ((0, s0), (1, s1)):
                nc.vector.tensor_tensor(
                    out=_ap(s, pc, 0, [(4, n), (2, 2), (1, 2)]),
                    in0=_ap(A2.tile, pc, A2.off + k * A2.dc,
                            [(A2.dm, n), (A2.dr, 2), (0, 2)]),
                    in1=_ap(B2.tile, pc, B2.off + k * B2.dr,
                            [(B2.dm, n), (0, 2), (B2.dc, 2)]),
                    op=AL.mult)
            nc.vector.tensor_tensor(
                out=_ap(store, pc, 0, [(1, n * 4)]),
                in0=_ap(s0, pc, 0, [(1, n * 4)]),
                in1=_ap(s1, pc, 0, [(1, n * 4)]),
                op=AL.add)
            return Mat(store, 0, 4, 2, 1)

        Psub = Mat(C.tile, C.off, C.dm, C.dr, C.dc)
        Qsub = Mat(C.tile, C.off + 2 * C.dc, C.dm, C.dr, C.dc)
        Ssub = Mat(C.tile, C.off + 2 * C.dr + 2 * C.dc, C.dm, C.dr, C.dc)
        pin = t(n * 4)
        Pinv = inv22(Psub, pin)
        at = t(n * 4)
        A2 = mm22(Pinv, Qsub, at)
        qta = t(n * 4)
        mm22(Qsub.T(), A2, qta)
        tt = t(n * 4)
        nc.vector.tensor_tensor(
            out=_ap(tt, pc, 0, [(4, n), (2, 2), (1, 2)]),
            in0=_ap(Ssub.tile, pc, Ssub.off, [(Ssub.dm, n), (Ssub.dr, 2), (Ssub.dc, 2)]),
            in1=_ap(qta, pc, 0, [(4, n), (2, 2), (1, 2)]),
            op=AL.subtract)
        tin = t(n * 4)
        Tinv = inv22(Mat(tt, 0, 4, 2, 1), tin)
        att = t(n * 4)
        AT = mm22(A2, Tinv, att)
        ataT = t(n * 4)
        mm22(AT, A2.T(), ataT)
        nc.vector.tensor_tensor(
            out=_ap(Mout, pc, 0, [(16, n), (4, 2), (1, 2)]),
            in0=_ap(pin, pc, 0, [(4, n), (2, 2), (1, 2)]),
            in1=_ap(ataT, pc, 0, [(4, n), (2, 2), (1, 2)]),
            op=AL.add)
        nc.vector.tensor_scalar(
            out=_ap(Mout, pc, 2, [(16, n), (4, 2), (1, 2)]),
            in0=_ap(att, pc, 0, [(4, n), (2, 2), (1, 2)]),
            scalar1=-1.0, scalar2=None, op0=AL.mult)
        nc.vector.tensor_scalar(
            out=_ap(Mout, pc, 8, [(16, n), (4, 2), (1, 2)]),
            in0=_ap(att, pc, 0, [(4, n), (1, 2), (2, 2)]),
            scalar1=-1.0, scalar2=None, op0=AL.mult)
        nc.vector.tensor_copy(
            _ap(Mout, pc, 10, [(16, n), (4, 2), (1, 2)]),
            _ap(tin, pc, 0, [(4, n), (2, 2), (1, 2)]))
        return Mat(Mout, 0, 16, 4, 1)


def _udescs(tile, canon, s16, n):
    if canon:
        return (Mat(tile, 0, 2 * s16, 4, 1), Mat(tile, s16, 2 * s16, 4, 1))
    # i-major compact from previous level's mm44 (2n entries)
    return (Mat(tile, 0, 8, 2 * n * 4, 1), Mat(tile, 4, 8, 2 * n * 4, 1))


def build():
    nc = bacc.Bacc("TRN2", target_bir_lowering=False, debug=False, num_devices=8)
    d = {}

    def din(name, shape):
        d[name] = nc.dram_tensor(name, shape, f32, kind="ExternalInput")
        return d[name]

    for name, shape in [
        ("th_pad", [BL, 130, 4]), ("q_pad", [BL, 129, 4]), ("wob", [BL, T1]),
        ("epst", [BL, T1]), ("startt", [P, 4]), ("goalt", [P, 4]),
        ("sdf", [BL, HS, WS]), ("negiota", [P, 1]), ("negiotap1", [P, 1]),
        ("sgn4d", [P, 4]), ("onesbd8", [P, 16]), ("ones1", [P, 1]),
        ("maskL0", [P, 1]), ("maskL7", [P, 1]), ("maskLn0", [P, 1]),
        ("constD", [P, 256]),
    ]:
        din(name, shape)

    dtheta = nc.dram_tensor("dtheta", [BL, T1, SD], f32, kind="ExternalOutput")
    errs = nc.dram_tensor("errs", [16, 2], f32, kind="ExternalOutput")

    scr = {k: nc.dram_tensor(f"scr_{k}", [2048], f32)
           for k in ["py", "y0", "px", "x0", "dist", "gx", "gy"]}
    scrF = nc.dram_tensor("scr_fold", [3 * 2048], f32)
    scrX = nc.dram_tensor("scr_x", [512], f32)

    with TileContext(nc) as tc:
        with tc.tile_pool(name="main", bufs=1) as pool, \
             tc.tile_pool(name="psA", bufs=3, space="PSUM") as psA, \
             tc.tile_pool(name="psR", bufs=1, space="PSUM") as psR:

            def ld(name, shape, ap_dims, off=0):
                t_ = _mt(pool, shape)
                nc.sync.dma_start(t_[:], _dap(d[name], off, ap_dims))
                return t_

            th = ld("th_pad", [P, 72], [(520, 16), (64, 8), (1, 72)])
            qx = ld("q_pad", [P, 68], [(516, 16), (64, 8), (1, 68)])
            wt = ld("wob", [P, 16], [(128, 16), (16, 8), (1, 16)])
            ept = ld("epst", [P, 16], [(128, 16), (16, 8), (1, 16)])
            stt = ld("startt", [P, 4], [(4, 128), (1, 4)])
            glt = ld("goalt", [P, 4], [(4, 128), (1, 4)])
            sdft = ld("sdf", [P, BL * WS], [(128, 128), (16384, 16), (1, 128)])
            ni = ld("negiota", [P, 1], [(1, 128), (1, 1)])
            nip = ld("negiotap1", [P, 1], [(1, 128), (1, 1)])
            sgn4 = ld("sgn4d", [P, 4], [(4, 128), (1, 4)])
            obd8 = ld("onesbd8", [P, 16], [(16, 128), (1, 16)])
            ones1t = ld("ones1", [P, 1], [(1, 128), (1, 1)])
            mkL0 = ld("maskL0", [P, 1], [(1, 128), (1, 1)])
            mkL7 = ld("maskL7", [P, 1], [(1, 128), (1, 1)])
            mkLn0 = ld("maskLn0", [P, 1], [(1, 128), (1, 1)])
            cD = ld("constD", [P, 256], [(256, 128), (1, 256)])

            TT = nc.vector.tensor_tensor
            TS = nc.vector.tensor_scalar
            STT = nc.vector.scalar_tensor_tensor
            TTR = nc.vector.tensor_tensor_reduce

            # ---------------- coordinates ----------------
            def thc(comp, cnt=16, k0=0):
                return _ap(th, P, (k0 + 1) * 4 + comp, [(4, cnt)])

            pxt = _mt(pool, [P, 16])
            pyt = _mt(pool, [P, 16])
            TS(out=pxt[:], in0=thc(0), scalar1=(WS - 1) / 2.0,
               scalar2=(WS - 1) / 2.0, op0=AL.mult, op1=AL.add)
            TS(out=pyt[:], in0=thc(1), scalar1=(HS - 1) / 2.0,
               scalar2=(HS - 1) / 2.0, op0=AL.mult, op1=AL.add)
            pxc = _mt(pool, [P, 16])
            pyc = _mt(pool, [P, 16])
            TS(out=pxc[:], in0=pxt[:], scalar1=0.0, scalar2=float(WS - 2),
               op0=AL.max, op1=AL.min)
            TS(out=pyc[:], in0=pyt[:], scalar1=0.0, scalar2=float(HS - 2),
               op0=AL.max, op1=AL.min)
            xi32 = _mt(pool, [P, 16], mybir.dt.int32)
            yi32 = _mt(pool, [P, 16], mybir.dt.int32)
            x0f = _mt(pool, [P, 16])
            y0f = _mt(pool, [P, 16])
            nc.vector.tensor_copy(xi32[:], pxc[:])
            nc.vector.tensor_copy(yi32[:], pyc[:])
            nc.vector.tensor_copy(x0f[:], xi32[:])
            nc.vector.tensor_copy(y0f[:], yi32[:])
            # HW cast rounds (CoreSim truncates): correct to exact floor
            gtx = _mt(pool, [P, 16])
            TT(out=gtx[:], in0=x0f[:], in1=pxc[:], op=AL.is_gt)
            TT(out=x0f[:], in0=x0f[:], in1=gtx[:], op=AL.subtract)
            gty = _mt(pool, [P, 16])
            TT(out=gty[:], in0=y0f[:], in1=pyc[:], op=AL.is_gt)
            TT(out=y0f[:], in0=y0f[:], in1=gty[:], op=AL.subtract)

            def fold_repl(srct, key, tag):
                nc.sync.dma_start(_dap(scr[key], 0, [(16, 128), (1, 16)]), srct[:])
                rt_ = _mt(pool, [P, 2048], tag=tag)
                nc.sync.dma_start(rt_[:], _dap(scr[key], 0, [(0, 128), (1, 2048)]))
                return rt_

            py_r = fold_repl(pyt, "py", "rp0")
            y0_r = fold_repl(y0f, "y0", "rp1")

            # ---------------- masks ----------------
            def actpair(src, biasap, fn1, tag=None):
                a1 = _mt(pool, [P, 2048], tag="actscr")
                nc.scalar.activation(a1[:], src[:], fn1, bias=biasap, scale=1.0)
                h = _mt(pool, [P, 2048], tag=tag)
                nc.scalar.activation(h[:], a1[:], AF.Relu, bias=1.0, scale=-1.0)
                return h

            hatY = actpair(py_r, ni[:], AF.Abs, tag="hatY")
            ohYlo = actpair(y0_r, ni[:], AF.Square, tag="oh0")
            hatYD = actpair(y0_r, nip[:], AF.Square, tag="hatYD")
            TT(out=hatYD[:], in0=hatYD[:], in1=ohYlo[:], op=AL.subtract)
            px_r = fold_repl(pxt, "px", "rp0")
            x0_r = fold_repl(x0f, "x0", "rp1")
            hatX = actpair(px_r, ni[:], AF.Abs, tag="hatX")
            ohXlo = actpair(x0_r, ni[:], AF.Square, tag="oh0")
            hatXD = actpair(x0_r, nip[:], AF.Square, tag="hatXD")
            TT(out=hatXD[:], in0=hatXD[:], in1=ohXlo[:], op=AL.subtract)

            # ---------------- per-traj interp matmuls ----------------
            prodD = _mt(pool, [P, 2048])
            prodGX = _mt(pool, [P, 2048])
            prodGY = _mt(pool, [P, 2048])
            for b in range(BL):
                sl = slice(b * 128, (b + 1) * 128)
                A_b = psA.tile([P, 128], f32, tag="A")
                AD_b = psA.tile([P, 128], f32, tag="AD")
                nc.tensor.matmul(A_b[:], lhsT=sdft[:, sl], rhs=hatY[:, sl],
                                 start=True, stop=True)
                nc.tensor.matmul(AD_b[:], lhsT=sdft[:, sl], rhs=hatYD[:, sl],
                                 start=True, stop=True)
                TT(out=prodD[:, sl], in0=A_b[:], in1=hatX[:, sl], op=AL.mult)
                TT(out=prodGX[:, sl], in0=A_b[:], in1=hatXD[:, sl], op=AL.mult)
                TT(out=prodGY[:, sl], in0=AD_b[:], in1=hatX[:, sl], op=AL.mult)

            for key, prod in (("dist", prodD), ("gx", prodGX), ("gy", prodGY)):
                for c in range(4):
                    sl = slice(c * 512, (c + 1) * 512)
                    rp = psR.tile([1, 512], f32, tag="R")
                    nc.tensor.matmul(rp[:], lhsT=ones1t[:], rhs=prod[:, sl],
                                     start=True, stop=True)
                    sb = _mt(pool, [1, 512])
                    nc.scalar.copy(sb[:], rp[:])
                    nc.sync.dma_start(_dap(scr[key], c * 512, [(1, 512)]), sb[0:1, :])
            dist = _mt(pool, [P, 16])
            gxs = _mt(pool, [P, 16])
            gys = _mt(pool, [P, 16])
            nc.sync.dma_start(dist[:], _dap(scr["dist"], 0, [(16, 128), (1, 16)]))
            nc.sync.dma_start(gxs[:], _dap(scr["gx"], 0, [(16, 128), (1, 16)]))
            nc.sync.dma_start(gys[:], _dap(scr["gy"], 0, [(16, 128), (1, 16)]))

            # ---------------- obstacle terms ----------------
            robs = _mt(pool, [P, 16])
            TT(out=robs[:], in0=ept[:], in1=dist[:], op=AL.subtract)
            TS(out=robs[:], in0=robs[:], scalar1=0.0, scalar2=None, op0=AL.max)
            actv = _mt(pool, [P, 16])
            TS(out=actv[:], in0=robs[:], scalar1=0.0, scalar2=None, op0=AL.is_gt)
            jt = _mt(pool, [P, 32])
            tmpj = _mt(pool, [P, 16])
            TT(out=tmpj[:], in0=actv[:], in1=gxs[:], op=AL.mult)
            TS(out=_ap(jt, P, 0, [(2, 16)]), in0=tmpj[:],
               scalar1=-(WS - 1) / 2.0, scalar2=None, op0=AL.mult)
            tmpj2 = _mt(pool, [P, 16])
            TT(out=tmpj2[:], in0=actv[:], in1=gys[:], op=AL.mult)
            TS(out=_ap(jt, P, 1, [(2, 16)]), in0=tmpj2[:],
               scalar1=-(HS - 1) / 2.0, scalar2=None, op0=AL.mult)
            wr = _mt(pool, [P, 16])
            TT(out=wr[:], in0=wt[:], in1=robs[:], op=AL.mult)

            # ---------------- assemble D, U, r ----------------
            Dt = _mt(pool, [P, 256])
            Ut = _mt(pool, [P, 256])
            rt = _mt(pool, [P, 64])
            for a in range(2):
                for c in range(2):
                    dsub = _ap(Dt, P, 8 * a + 2 * c, [(16, 16), (4, 2), (1, 2)])
                    qt_ap = _ap(qx, P, 4, [(4, 16), (2, 2), (1, 2)])
                    qp_ap = _ap(qx, P, 0, [(4, 16), (2, 2), (1, 2)])
                    TS(out=dsub, in0=qt_ap, scalar1=float(PSP[a, c]),
                       scalar2=None, op0=AL.mult)
                    qps = _mt(pool, [P, 64], tag="qps")
                    TS(out=_ap(qps, P, 0, [(4, 16), (1, 4)]),
                       in0=_ap(qx, P, 0, [(4, 16), (1, 4)]),
                       scalar1=float(SM[a, c]), scalar2=None, op0=AL.mult)
                    TT(out=dsub, in0=dsub,
                       in1=_ap(qps, P, 0, [(4, 16), (2, 2), (1, 2)]), op=AL.add)
                    usub = _ap(Ut, P, 8 * a + 2 * c, [(16, 16), (4, 2), (1, 2)])
                    TS(out=usub, in0=qt_ap, scalar1=float(-PTS[a, c]),
                       scalar2=None, op0=AL.mult)
            TT(out=Dt[:], in0=Dt[:], in1=cD[:], op=AL.add)
            jj = _mt(pool, [P, 64])
            TT(out=_ap(jj, P, 0, [(4, 16), (2, 2), (1, 2)]),
               in0=_ap(jt, P, 0, [(2, 16), (1, 2), (0, 2)]),
               in1=_ap(jt, P, 0, [(2, 16), (0, 2), (1, 2)]), op=AL.mult)
            TT(out=_ap(jj, P, 0, [(4, 16), (1, 4)]),
               in0=_ap(jj, P, 0, [(4, 16), (1, 4)]),
               in1=_ap(wt, P, 0, [(1, 16), (0, 4)]), op=AL.mult)
            dpos = _ap(Dt, P, 0, [(16, 16), (4, 2), (1, 2)])
            TT(out=dpos, in0=dpos, in1=_ap(jj, P, 0, [(4, 16), (2, 2), (1, 2)]),
               op=AL.add)

            rge = _mt(pool, [P, 68])
            nc.vector.tensor_copy(_ap(rge, P, 0, [(1, 68)]), _ap(th, P, 0, [(1, 68)]))
            STT(out=_ap(rge, P, 0, [(4, 17), (1, 2)]),
                in0=_ap(th, P, 2, [(4, 17), (1, 2)]), scalar=DT,
                in1=_ap(rge, P, 0, [(4, 17), (1, 2)]), op0=AL.mult, op1=AL.add)
            TT(out=_ap(rge, P, 0, [(1, 68)]), in0=_ap(rge, P, 0, [(1, 68)]),
               in1=_ap(th, P, 4, [(1, 68)]), op=AL.subtract)
            qr = _mt(pool, [P, 68])
            qscr = _mt(pool, [P, 68])
            for c in range(2):
                TT(out=_ap(qscr, P, 0, [(4, 17), (2, 2), (1, 2)]),
                   in0=_ap(qx, P, 0, [(4, 17), (2, 2), (1, 2)]),
                   in1=_ap(rge, P, 2 * c, [(4, 17), (0, 2), (1, 2)]),
                   op=AL.mult)
                nc.vector.tensor_reduce(
                    _ap(qr, P, 2 * c, [(4, 17), (1, 2)]),
                    _ap(qscr, P, 0, [(2, 34), (1, 2)]),
                    mybir.AxisListType.X, AL.add)
            nc.vector.memset(rt[:], 0.0)
            for a in range(2):
                for c in range(2):
                    rsub = _ap(rt, P, 2 * a, [(4, 16), (1, 2)])
                    STT(out=rsub, in0=_ap(qr, P, 4 + 2 * c, [(4, 16), (1, 2)]),
                        scalar=float(-PTS[a, c]), in1=rsub, op0=AL.mult, op1=AL.add)
                    STT(out=rsub, in0=_ap(qr, P, 2 * c, [(4, 16), (1, 2)]),
                        scalar=float(SM[a, c]), in1=rsub, op0=AL.mult, op1=AL.add)
            oscr = _mt(pool, [P, 32])
            TT(out=_ap(oscr, P, 0, [(2, 16), (1, 2)]),
               in0=_ap(wr, P, 0, [(1, 16), (0, 2)]),
               in1=_ap(jt, P, 0, [(2, 16), (1, 2)]), op=AL.mult)
            rsub0 = _ap(rt, P, 0, [(4, 16), (1, 2)])
            TT(out=rsub0, in0=rsub0, in1=_ap(oscr, P, 0, [(2, 16), (1, 2)]),
               op=AL.subtract)
            pscr = _mt(pool, [P, 4])
            pscr2 = _mt(pool, [P, 4])
            TT(out=pscr[:], in0=_ap(th, P, 4, [(1, 4)]), in1=stt[:], op=AL.subtract)
            TT(out=pscr[:], in0=pscr[:], in1=_ap(mkL0, P, 0, [(1, 1), (0, 4)]),
               op=AL.mult)
            STT(out=_ap(rt, P, 0, [(1, 4)]), in0=pscr[:], scalar=-KSI,
                in1=_ap(rt, P, 0, [(1, 4)]), op0=AL.mult, op1=AL.add)
            TT(out=pscr2[:], in0=_ap(th, P, 64, [(1, 4)]), in1=glt[:], op=AL.subtract)
            TT(out=pscr2[:], in0=pscr2[:], in1=_ap(mkL7, P, 0, [(1, 1), (0, 4)]),
               op=AL.mult)
            STT(out=_ap(rt, P, 60, [(1, 4)]), in0=pscr2[:], scalar=-KSI,
                in1=_ap(rt, P, 60, [(1, 4)]), op0=AL.mult, op1=AL.add)

            # ---------------- errors ----------------
            def redsum(srcap, width, scale=None, acc=None):
                red = _mt(pool, [P, 1])
                nc.vector.tensor_reduce(red[:], srcap, mybir.AxisListType.X, AL.add)
                if acc is None and scale is None:
                    return red
                na = _mt(pool, [P, 1])
                if scale is None:
                    TT(out=na[:], in0=red[:], in1=acc[:], op=AL.add)
                elif acc is None:
                    TS(out=na[:], in0=red[:], scalar1=scale, scalar2=None,
                       op0=AL.mult)
                else:
                    STT(out=na[:], in0=red[:], scalar=scale, in1=acc[:],
                        op0=AL.mult, op1=AL.add)
                return na

            acc = None
            egsc = _mt(pool, [P, 64])
            for a in range(2):
                TT(out=_ap(egsc, P, 0, [(4, 16), (2, 2), (1, 2)]),
                   in0=_ap(rge, P, 4 + 2 * a, [(4, 16), (0, 2), (1, 2)]),
                   in1=_ap(qr, P, 4, [(4, 16), (2, 2), (1, 2)]), op=AL.mult)
                for c in range(2):
                    sc2 = _mt(pool, [P, 32], tag="ersc")
                    TS(out=sc2[:], in0=_ap(egsc, P, 2 * c, [(4, 16), (1, 2)]),
                       scalar1=float(SM[a, c]), scalar2=None, op0=AL.mult)
                    acc = redsum(sc2[:], 32, acc=acc) if acc is not None else                         redsum(sc2[:], 32)
            eo = _mt(pool, [P, 16], tag="ersc")
            TT(out=eo[:], in0=wr[:], in1=robs[:], op=AL.mult)
            acc = redsum(eo[:], 16, acc=acc)
            ep1 = _mt(pool, [P, 4], tag="ersc")
            TT(out=ep1[:], in0=pscr[:], in1=pscr[:], op=AL.mult)
            acc = redsum(ep1[:], 4, scale=KSI, acc=acc)
            ep2 = _mt(pool, [P, 4], tag="ersc")
            TT(out=ep2[:], in0=pscr2[:], in1=pscr2[:], op=AL.mult)
            a3 = redsum(ep2[:], 4, scale=KSI, acc=acc)
            ex2 = _mt(pool, [P, 16], tag="ersc")
            TT(out=ex2[:], in0=robs[:], in1=robs[:], op=AL.mult)
            aex = redsum(ex2[:], 16)
            epart = _mt(pool, [P, 2])
            nc.vector.tensor_copy(_ap(epart, P, 0, [(1, 1)]), a3[:])
            nc.vector.tensor_copy(_ap(epart, P, 1, [(1, 1)]), aex[:])
            ep = psR.tile([16, 2], f32, tag="E")
            nc.tensor.matmul(ep[:], lhsT=obd8[:], rhs=epart[:],
                             start=True, stop=True)
            esb = _mt(pool, [16, 2])
            nc.scalar.copy(esb[:], ep[:])
            nc.sync.dma_start(errs[:], esb[:])

            # ---------------- CR solve: down, in-lane levels 0..3 ----------
            em = Emit(nc, pool, P)
            lane_lvls = []
            Ucur_tile, Ucur_canon = Ut, True
            for l in range(4):
                s = 1 << l
                n = 8 >> l
                em.n = n
                s16 = s * 16
                Ul, Ur = _udescs(Ucur_tile, Ucur_canon, s16, n)
                Codd = Mat(Dt, s16, 2 * s16, 4, 1)
                Mlt = _mt(pool, [P, n * 16])
                M = em.inv44(Codd, Mlt, sgn4)
                W = em.mm44(M, Ur)
                V = em.mm44(M, Ul.T())
                dDl = em.mm44(Ul, V)
                dDr = em.mm44(Ur.T(), W)
                Un = em.mm44(Ul, W, negate=True, tag=f"unl{l}")
                mr = em.mv4(M, Vec(rt, s * 4, 2 * s * 4, 1))
                drl = em.mv4(Ul, mr)
                drr = em.mv4(Ur.T(), mr)
                lane_lvls.append((s, n, M, Ul, Ur))
                TT(out=_ap(Dt, P, 0, [(2 * s16, n), (4, 4), (1, 4)]),
                   in0=_ap(Dt, P, 0, [(2 * s16, n), (4, 4), (1, 4)]),
                   in1=_ap(dDl.tile, P, 0, [(4, n), (n * 4, 4), (1, 4)]),
                   op=AL.subtract)
                TT(out=_ap(rt, P, 0, [(2 * s * 4, n), (1, 4)]),
                   in0=_ap(rt, P, 0, [(2 * s * 4, n), (1, 4)]),
                   in1=_ap(drl.tile, P, 0, [(4, n), (1, 4)]),
                   op=AL.subtract)
                if n > 1:
                    TT(out=_ap(Dt, P, 2 * s16, [(2 * s16, n - 1), (4, 4), (1, 4)]),
                       in0=_ap(Dt, P, 2 * s16, [(2 * s16, n - 1), (4, 4), (1, 4)]),
                       in1=_ap(dDr.tile, P, 0, [(4, n - 1), (n * 4, 4), (1, 4)]),
                       op=AL.subtract)
                    TT(out=_ap(rt, P, 2 * s * 4, [(2 * s * 4, n - 1), (1, 4)]),
                       in0=_ap(rt, P, 2 * s * 4, [(2 * s * 4, n - 1), (1, 4)]),
                       in1=_ap(drr.tile, P, 0, [(4, n - 1), (1, 4)]),
                       op=AL.subtract)
                ex = _mt(pool, [P, 20])
                nc.vector.tensor_copy(
                    _ap(ex, P, 0, [(4, 4), (1, 4)]),
                    _ap(dDr.tile, P, (n - 1) * 4, [(n * 4, 4), (1, 4)]))
                nc.vector.tensor_copy(
                    _ap(ex, P, 16, [(1, 4)]),
                    _ap(drr.tile, P, (n - 1) * 4, [(1, 4)]))
                imp = _mt(pool, [P, 20])
                nc.vector.memset(imp[:], 0.0)
                nc.sync.dma_start(imp[1:128, :], ex[0:127, :])
                TT(out=imp[:], in0=imp[:],
                   in1=_ap(mkLn0, P, 0, [(1, 1), (0, 20)]), op=AL.mult)
                TT(out=_ap(Dt, P, 0, [(1, 16)]), in0=_ap(Dt, P, 0, [(1, 16)]),
                   in1=_ap(imp, P, 0, [(1, 16)]), op=AL.subtract)
                TT(out=_ap(rt, P, 0, [(1, 4)]), in0=_ap(rt, P, 0, [(1, 4)]),
                   in1=_ap(imp, P, 16, [(1, 4)]), op=AL.subtract)
                Ucur_tile, Ucur_canon = Un.tile, False

            # ---------------- fold to 16 partitions ----------------
            nc.sync.dma_start(_dap(scrF, 0, [(16, 128), (1, 16)]),
                              _ap(Dt, P, 0, [(1, 16)]))
            nc.sync.dma_start(_dap(scrF, 2048, [(16, 128), (1, 16)]),
                              _ap(Ucur_tile, P, 0, [(1, 16)]))
            nc.sync.dma_start(_dap(scrF, 4096, [(4, 128), (1, 4)]),
                              _ap(rt, P, 0, [(1, 4)]))
            Df = _mt(pool, [P, 144])
            Uf = _mt(pool, [P, 144])
            rf = _mt(pool, [P, 36])
            nc.vector.memset(Df[0:16, :], 0.0)
            nc.vector.memset(Uf[0:16, :], 0.0)
            nc.vector.memset(rf[0:16, :], 0.0)
            nc.sync.dma_start(Df[0:16, 0:128], _dap(scrF, 0, [(128, 16), (1, 128)]))
            nc.sync.dma_start(Uf[0:16, 0:128], _dap(scrF, 2048, [(128, 16), (1, 128)]))
            nc.sync.dma_start(rf[0:16, 0:32], _dap(scrF, 4096, [(32, 16), (1, 32)]))

            # ---------------- in-free levels 4..6 ----------------
            emf = Emit(nc, pool, 16)
            free_lvls = []
            Ucur_tile, Ucur_canon = Uf, True
            for l4 in range(3):
                s = 1 << l4
                n = 4 >> l4
                emf.n = n
                s16 = s * 16
                Ul, Ur = _udescs(Ucur_tile, Ucur_canon, s16, n)
                Codd = Mat(Df, s16, 2 * s16, 4, 1)
                Mlt = _mt(pool, [P, n * 16])
                M = emf.inv44(Codd, Mlt, sgn4)
                W = emf.mm44(M, Ur)
                V = emf.mm44(M, Ul.T())
                dDl = emf.mm44(Ul, V)
                dDr = emf.mm44(Ur.T(), W)
                Un = emf.mm44(Ul, W, negate=True, tag=f"unf{l4}")
                mr = emf.mv4(M, Vec(rf, s * 4, 2 * s * 4, 1))
                drl = emf.mv4(Ul, mr)
                drr = emf.mv4(Ur.T(), mr)
                free_lvls.append((s, n, M, Ul, Ur))
                TT(out=_ap(Df, 16, 0, [(2 * s16, n), (4, 4), (1, 4)]),
                   in0=_ap(Df, 16, 0, [(2 * s16, n), (4, 4), (1, 4)]),
                   in1=_ap(dDl.tile, 16, 0, [(4, n), (n * 4, 4), (1, 4)]),
                   op=AL.subtract)
                TT(out=_ap(rf, 16, 0, [(2 * s * 4, n), (1, 4)]),
                   in0=_ap(rf, 16, 0, [(2 * s * 4, n), (1, 4)]),
                   in1=_ap(drl.tile, 16, 0, [(4, n), (1, 4)]),
                   op=AL.subtract)
                TT(out=_ap(Df, 16, 2 * s16, [(2 * s16, n), (4, 4), (1, 4)]),
                   in0=_ap(Df, 16, 2 * s16, [(2 * s16, n), (4, 4), (1, 4)]),
                   in1=_ap(dDr.tile, 16, 0, [(4, n), (n * 4, 4), (1, 4)]),
                   op=AL.subtract)
                TT(out=_ap(rf, 16, 2 * s * 4, [(2 * s * 4, n), (1, 4)]),
                   in0=_ap(rf, 16, 2 * s * 4, [(2 * s * 4, n), (1, 4)]),
                   in1=_ap(drr.tile, 16, 0, [(4, n), (1, 4)]),
                   op=AL.subtract)
                Ucur_tile, Ucur_canon = Un.tile, False

            # ---------------- peak + up in-free ----------------
            emf.n = 1
            Mpk = _mt(pool, [P, 16])
            Mp = emf.inv44(Mat(Df, 0, 16, 4, 1), Mpk, sgn4)
            xf = _mt(pool, [P, 36])
            nc.vector.memset(xf[0:16, :], 0.0)
            emf.mv4(Mp, Vec(rf, 0, 4, 1), out=_ap(xf, 16, 0, [(1, 4)]))
            for l4 in range(2, -1, -1):
                s, n, M, Ul, Ur = free_lvls[l4]
                emf.n = n
                t1 = emf.mv4(Ul.T(), Vec(xf, 0, 2 * s * 4, 1))
                t2 = emf.mv4(Ur, Vec(xf, 2 * s * 4, 2 * s * 4, 1))
                tv = _mt(pool, [P, n * 4])
                TT(out=_ap(tv, 16, 0, [(1, n * 4)]),
                   in0=_ap(rf, 16, s * 4, [(2 * s * 4, n), (1, 4)]),
                   in1=_ap(t1.tile, 16, 0, [(1, n * 4)]), op=AL.subtract)
                TT(out=_ap(tv, 16, 0, [(1, n * 4)]),
                   in0=_ap(tv, 16, 0, [(1, n * 4)]),
                   in1=_ap(t2.tile, 16, 0, [(1, n * 4)]), op=AL.subtract)
                emf.mv4(M, Vec(tv, 0, 4, 1),
                        out=_ap(xf, 16, s * 4, [(2 * s * 4, n), (1, 4)]))

            # unfold xf -> x block0 positions
            xt = _mt(pool, [P, 64])
            nc.sync.dma_start(_dap(scrX, 0, [(32, 16), (1, 32)]), xf[0:16, 0:32])
            nc.sync.dma_start(_ap(xt, P, 0, [(1, 4)]),
                              _dap(scrX, 0, [(4, 128), (1, 4)]))

            # ---------------- up in-lane 3..0 ----------------
            ximp = _mt(pool, [P, 4])
            nc.vector.memset(ximp[:], 0.0)
            nc.sync.dma_start(ximp[0:127, :], xt[1:128, 0:4])
            for l in range(3, -1, -1):
                s, n, M, Ul, Ur = lane_lvls[l]
                em.n = n
                t1 = em.mv4(Ul.T(), Vec(xt, 0, 2 * s * 4, 1))
                xsr = _mt(pool, [P, n * 4])
                if n > 1:
                    nc.vector.tensor_copy(
                        _ap(xsr, P, 0, [(4, n - 1), (1, 4)]),
                        _ap(xt, P, 2 * s * 4, [(2 * s * 4, n - 1), (1, 4)]))
                nc.vector.tensor_copy(_ap(xsr, P, (n - 1) * 4, [(1, 4)]), ximp[:])
                t2 = em.mv4(Ur, Vec(xsr, 0, 4, 1))
                tv = _mt(pool, [P, n * 4])
                TT(out=_ap(tv, P, 0, [(1, n * 4)]),
                   in0=_ap(rt, P, s * 4, [(2 * s * 4, n), (1, 4)]),
                   in1=_ap(t1.tile, P, 0, [(1, n * 4)]), op=AL.subtract)
                TT(out=_ap(tv, P, 0, [(1, n * 4)]),
                   in0=_ap(tv, P, 0, [(1, n * 4)]),
                   in1=_ap(t2.tile, P, 0, [(1, n * 4)]), op=AL.subtract)
                em.mv4(M, Vec(tv, 0, 4, 1),
                       out=_ap(xt, P, s * 4, [(2 * s * 4, n), (1, 4)]))

            # ---------------- output ----------------
            nc.sync.dma_start(_dap(dtheta, 0, [(64, 128), (1, 64)]), xt[:])

    nc.compile()
    return nc


_NC_CACHE = None


def _get_nc():
    global _NC_CACHE
    if _NC_CACHE is None:
        _NC_CACHE = build()
    return _NC_CACHE


def _consts():
    c = {}
    c["negiota"] = -np.arange(P, dtype=np.float32).reshape(P, 1)
    c["negiotap1"] = c["negiota"] + 1.0
    c["sgn4d"] = np.tile(np.array([1.0, -1.0, -1.0, 1.0], np.float32), (P, 1))
    obd = np.zeros((P, 16), np.float32)
    for g in range(16):
        obd[8 * g:8 * (g + 1), g] = 1.0
    c["onesbd8"] = obd
    c["ones1"] = np.ones((P, 1), np.float32)
    L = np.arange(P) % 8
    c["maskL0"] = (L == 0).astype(np.float32).reshape(P, 1)
    c["maskL7"] = (L == 7).astype(np.float32).reshape(P, 1)
    c["maskLn0"] = (L != 0).astype(np.float32).reshape(P, 1)
    cd = np.zeros((P, 256), np.float32)
    eye = np.eye(4, dtype=np.float32).ravel()
    for k in range(16):
        cd[:, 16 * k:16 * (k + 1)] += REG * eye
    cd[L == 0, 0:16] += KSI * eye
    cd[L == 7, 240:256] += KSI * eye
    c["constD"] = cd
    return c


def _stage(core, thb, startb, goalb, sdfb, qc, wob, eps):
    sl = slice(core * BL, (core + 1) * BL)
    th = np.ascontiguousarray(thb[sl]).astype(np.float32)
    th_pad = np.zeros((BL, 130, 4), np.float32)
    th_pad[:, 1:129] = th
    q = np.ascontiguousarray(qc[sl]).astype(np.float32).reshape(BL, NGP, 4)
    q_pad = np.zeros((BL, 129, 4), np.float32)
    q_pad[:, 1:128] = q
    m = {
        "th_pad": th_pad,
        "q_pad": q_pad,
        "wob": np.ascontiguousarray(wob[sl]).astype(np.float32),
        "epst": np.ascontiguousarray(eps[sl]).astype(np.float32),
        "startt": np.repeat(startb[sl, 0].astype(np.float32), 8, axis=0),
        "goalt": np.repeat(goalb[sl, 0].astype(np.float32), 8, axis=0),
        "sdf": np.ascontiguousarray(sdfb[sl]).astype(np.float32),
    }
    return m


def kernel(thb, startb, goalb, imb, sdfb, qc_inv_trajb, obscov_inv_trajb,
           eps_trajb):
    from concourse.bass_utils import run_bass_kernel_spmd
    thb = np.asarray(thb, np.float32)
    startb = np.asarray(startb, np.float32)
    goalb = np.asarray(goalb, np.float32)
    sdfb = np.asarray(sdfb, np.float32)
    qc = np.asarray(qc_inv_trajb, np.float32)
    wob = np.asarray(obscov_inv_trajb, np.float32)[:, :, 0, 0]
    eps = np.asarray(eps_trajb, np.float32)[:, :, 0]
    nc = _get_nc()
    consts = _consts()
    in_maps = []
    for core in range(8):
        m = _stage(core, thb, startb, goalb, sdfb, qc, wob, eps)
        m.update(consts)
        in_maps.append(m)
    res = run_bass_kernel_spmd(nc, in_maps, list(range(8))).results
    B = thb.shape[0]
    dtheta = np.concatenate([res[c]["dtheta"] for c in range(8)], axis=0)
    errs = np.concatenate([res[c]["errs"] for c in range(8)], axis=0)
    return dtheta.reshape(B, T1, SD), errs[:, 0].copy(), errs[:, 1].copy()
